# revision 9
# baseline (speedup 1.0000x reference)
"""Trainium2 Bass kernel for nn_EntropyLoss (256-bin histogram entropy diff).

Strategy (data-parallel over 8 NeuronCores, per sharding hint):
  - Each core takes 8 of the 64 batch entries of both tensors (4,194,304
    f32 elements per tensor per core, laid out [128, 32768]).
  - On-device, bit-exact replication of the reference binning
    idx = floor((x + 1.0f) * 128.0f):
      u = (x + 1)*128            (dual-op tensor_scalar, fp32 — same rounding
                                  as the reference's (x - lo)/width since the
                                  divisor is an exact power of two)
      r = int16(u)               (hardware cast, round-half-even)
      m = (u < r)                (round went up)
      j = r - m                  = floor(u), exact for every fp32 input
    Bin counts = 256 tensor_scalar(is_equal k, accum_out) passes over j.
    Values outside [-1, 1] give j outside 0..255 and are never counted;
    x == 1.0 (u == 256.0 exactly) belongs in bin 255 per torch.histc and is
    counted by one extra fp32 is_equal(256.0) pass per prep sub-chunk.
  - Host: sum the exact per-core/per-partition/per-chunk integer counts, then
    compute both entropies and the abs diff with float32 jax ops on CPU,
    mirroring the reference formula op-for-op.
"""

import numpy as np

B, C, H, W = 64, 2, 512, 512
N_CORES = 8
P = 128
ELEMS_PER_CORE = (B // N_CORES) * C * H * W            # 4,194,304
FREE = ELEMS_PER_CORE // P                             # 32,768
F_CHUNK = 8192                                         # counting chunk (free dim)
N_CHUNKS = FREE // F_CHUNK                             # 4 per tensor
F_SUB = 2048                                           # prep sub-chunk
N_SUB = F_CHUNK // F_SUB                               # 4
TOT_CHUNKS = 2 * N_CHUNKS                              # pred chunks 0..3, gt 4..7
NB = 256
COL_EXTRA = TOT_CHUNKS * NB                            # 2048: u==256.0 tallies
COL_EXTRA2 = COL_EXTRA + TOT_CHUNKS * N_SUB            # 2080: x==1+2^-23 tallies
NCOL = COL_EXTRA2 + TOT_CHUNKS * N_SUB                 # 2112
EPS = 1e-8
X_ABOVE_ONE = float(np.float32(1.0) + np.float32(2.0 ** -23))

_CACHE = {}


def _build():
    import concourse.bacc as bacc
    import concourse.mybir as mybir
    import concourse.tile as tile

    f32 = mybir.dt.float32
    i16 = mybir.dt.int16
    bf16 = mybir.dt.bfloat16
    op = mybir.AluOpType

    nc = bacc.Bacc("TRN2", target_bir_lowering=False, debug=False,
                   num_devices=N_CORES)
    pred_d = nc.dram_tensor("pred", [P, FREE], f32, kind="ExternalInput")
    gt_d = nc.dram_tensor("gt", [P, FREE], f32, kind="ExternalInput")
    out_d = nc.dram_tensor("o", [P, NCOL], f32, kind="ExternalOutput")

    with tile.TileContext(nc) as tc:
        with (
            tc.tile_pool(name="xp", bufs=3) as xpool,
            tc.tile_pool(name="up", bufs=2) as upool,
            tc.tile_pool(name="jp", bufs=2) as jpool,
            tc.tile_pool(name="tp", bufs=3) as tpool,
            tc.tile_pool(name="sp", bufs=2) as spool,
            tc.tile_pool(name="ap", bufs=1) as apool,
        ):
            acc = apool.tile([P, NCOL], f32)
            for t_i, src in ((0, pred_d), (1, gt_d)):
                for q in range(N_CHUNKS):
                    c = t_i * N_CHUNKS + q
                    j2 = jpool.tile([P, F_CHUNK], i16, tag="j2")
                    for s in range(N_SUB):
                        lo = q * F_CHUNK + s * F_SUB
                        sl = slice(s * F_SUB, (s + 1) * F_SUB)
                        x_sub = xpool.tile([P, F_SUB], f32, tag="x")
                        nc.sync.dma_start(x_sub[:], src.ap()[:, lo:lo + F_SUB])
                        u_sub = upool.tile([P, F_SUB], f32, tag="u")
                        nc.vector.tensor_scalar(
                            u_sub[:], x_sub[:], 1.0, 128.0, op.add, op.mult)
                        r_sub = upool.tile([P, F_SUB], i16, tag="r")
                        nc.vector.tensor_copy(r_sub[:], u_sub[:])
                        m_sub = upool.tile([P, F_SUB], i16, tag="m")
                        nc.vector.tensor_tensor(m_sub[:], u_sub[:], r_sub[:], op.is_lt)
                        nc.vector.tensor_tensor(j2[:, sl], r_sub[:], m_sub[:], op.subtract)
                        # u == 256.0 detector (x in {0.99999994, 1.0, 1+2^-23})
                        t5 = spool.tile([P, F_SUB], bf16, tag="t5")
                        nc.vector.tensor_scalar(
                            t5[:], u_sub[:], 256.0, None, op.is_equal, op.add,
                            accum_out=acc[:, COL_EXTRA + c * N_SUB + s:
                                          COL_EXTRA + c * N_SUB + s + 1])
                        # x == 1+2^-23 (the only x > 1 with u == 256.0; the
                        # reference's in_range mask excludes it)
                        t6 = spool.tile([P, F_SUB], bf16, tag="t6")
                        nc.vector.tensor_scalar(
                            t6[:], x_sub[:], X_ABOVE_ONE, None, op.is_equal, op.add,
                            accum_out=acc[:, COL_EXTRA2 + c * N_SUB + s:
                                          COL_EXTRA2 + c * N_SUB + s + 1])
                    for k in range(NB):
                        trash = tpool.tile([P, F_CHUNK], i16, tag="trash")
                        nc.vector.tensor_scalar(
                            trash[:], j2[:], float(k), None,
                            op.is_equal, op.add,
                            accum_out=acc[:, c * NB + k:c * NB + k + 1])
            nc.sync.dma_start(out_d.ap(), acc[:])
    nc.compile()
    return nc


def _get_nc():
    if "nc" not in _CACHE:
        _CACHE["nc"] = _build()
    return _CACHE["nc"]


def _shard(arr):
    """[64, 2, 512, 512] f32 -> list of 8 per-core [128, 32768] arrays."""
    a = np.ascontiguousarray(np.asarray(arr, dtype=np.float32))
    per = B // N_CORES
    return [a[i * per:(i + 1) * per].reshape(P, FREE) for i in range(N_CORES)]


def _entropy_diff_from_hists(hp, hg):
    """Mirror reference._entropy in float32 on CPU via jax."""
    import jax
    import jax.numpy as jnp

    cpu = jax.devices("cpu")[0]
    with jax.default_device(cpu):
        def ent(h):
            h = jnp.asarray(np.asarray(h, dtype=np.float32))
            prob = h / jnp.sum(h) + np.float32(EPS)
            return -jnp.sum(prob * jnp.log(prob))
        out = jnp.abs(ent(hp) - ent(hg))
        return np.asarray(out).astype(np.float32).reshape(())


def kernel(predicted_ab, ground_truth_ab):
    from concourse import bass_utils

    nc = _get_nc()
    preds = _shard(predicted_ab)
    gts = _shard(ground_truth_ab)
    in_maps = [{"pred": preds[i], "gt": gts[i]} for i in range(N_CORES)]
    res = bass_utils.run_bass_kernel_spmd(nc, in_maps, core_ids=list(range(N_CORES)))

    hist = np.zeros((2, NB), dtype=np.int64)
    extra = np.zeros(2, dtype=np.int64)
    extra2 = np.zeros(2, dtype=np.int64)
    for cidx in range(N_CORES):
        o = np.asarray(res.results[cidx]["o"], dtype=np.float64)
        for t in range(2):
            for q in range(N_CHUNKS):
                c = t * N_CHUNKS + q
                hist[t] += o[:, c * NB:(c + 1) * NB].sum(axis=0).round().astype(np.int64)
                extra[t] += int(o[:, COL_EXTRA + c * N_SUB:
                                  COL_EXTRA + (c + 1) * N_SUB].sum().round())
                extra2[t] += int(o[:, COL_EXTRA2 + c * N_SUB:
                                   COL_EXTRA2 + (c + 1) * N_SUB].sum().round())
    # u == 256.0: x in {0.99999994, 1.0} are in-range, floor 256 clips to bin
    # 255 in the reference; x == 1+2^-23 also lands on u == 256.0 but fails
    # the reference's x <= 1 mask, so subtract those.
    hist[0, NB - 1] += extra[0] - extra2[0]
    hist[1, NB - 1] += extra[1] - extra2[1]
    return _entropy_diff_from_hists(hist[0], hist[1])


if __name__ == "__main__":
    rng = np.random.default_rng(0)
    p = rng.standard_normal((B, C, H, W)).astype(np.float32)
    g = rng.standard_normal((B, C, H, W)).astype(np.float32)
    got = kernel(p, g)

    def host_hist(x):
        x = x.ravel()
        u = (x.astype(np.float32) + np.float32(1.0)) * np.float32(128.0)
        idx = np.clip(np.floor(u.astype(np.float64)).astype(np.int64), 0, 255)
        m = (x >= -1.0) & (x <= 1.0)
        return np.bincount(idx[m], minlength=256)

    hp, hg = host_hist(p), host_hist(g)
    exp = _entropy_diff_from_hists(hp, hg)
    print("kernel:", got, "host:", exp, "absdiff:", abs(float(got) - float(exp)))
